# revision 1
# baseline (speedup 1.0000x reference)
"""Efficient Channel Attention kernel for 8 Trainium2 NeuronCores.

Problem (B=4, N=4096, C=1024, H=4, HD=256):
    qkv = x @ Wqkv.T                 -> q,k,v per head, [HD, N] layout
    q,k l2-normalized over N; scores = (q*temp) @ k.T   [HD, HD] per (b,h)
    attn = softmax(scores, -1); out = attn @ v; y = out @ Wproj.T + bproj + x

Sharding: core = (batch b, token-half). All channel contractions are local;
the only cross-core coupling is the token(N)-contracted quantities: the raw
Grams k^T q and the q/k squared norms, AllReduce'd (~1MB) within the core
pair sharing a batch. Device layouts are channel-major (transposed); the
host feeds x^T / W^T slices and transposes the returned y^T back.

SBUF/PSUM pool tags are reused across phases (static pool allocation):
  wgt w0-7   : Wqkv^T qk-cols -> Wqkv^T v-cols -> Wproj^T
  xs  xs0-7  : x^T stream (A1) -> x^T stream (A2) -> B scratch / y + residual
  vo  vo0-8  : v chunks -> out^T chunks
  PSUM pA-pD : q/k accum -> sumsq rows -> v accum -> spm/atp -> proj accum
  PSUM pE,pF : Gram accumulators (2 heads each) -> attn@v accum
"""

import numpy as np

B, N, C, H = 4, 4096, 1024, 4
HD = C // H          # 256
NCORES = 8
NL = N // 2          # 2048 tokens per core
KT = C // 128        # 8 channel k-tiles
NT5 = NL // 512      # 4 token super-tiles
EPS = 1e-12

_CACHE = {}


def _build():
    import concourse.mybir as mybir
    import concourse.tile as tile
    from concourse import bacc
    from concourse.masks import make_identity

    f32 = mybir.dt.float32
    f32r = mybir.dt.float32r
    AX = mybir.AxisListType.X
    ADD = mybir.AluOpType.add
    Exp = mybir.ActivationFunctionType.Exp
    Ident = mybir.ActivationFunctionType.Identity

    nc = bacc.Bacc("TRN2", target_bir_lowering=False, debug=False,
                   num_devices=NCORES)

    xT_d = nc.dram_tensor("xT", [C, NL], f32r, kind="ExternalInput").ap()
    wqkT_d = nc.dram_tensor("wqkT", [C, 2 * C], f32r, kind="ExternalInput").ap()
    wvT_d = nc.dram_tensor("wvT", [C, C], f32r, kind="ExternalInput").ap()
    wpT_d = nc.dram_tensor("wpT", [C, C], f32r, kind="ExternalInput").ap()
    bias_d = nc.dram_tensor("bias", [128, KT], f32, kind="ExternalInput").ap()
    tmpv_d = nc.dram_tensor("tmpv", [128, KT], f32, kind="ExternalInput").ap()
    xrT_d = nc.dram_tensor("xrT", [C, NL], f32r, kind="ExternalInput").ap()
    yT_d = nc.dram_tensor("yT", [C, NL], f32, kind="ExternalOutput").ap()

    with tile.TileContext(nc) as tc:
        with (
            tc.tile_pool(name="const", bufs=1) as constp,
            tc.tile_pool(name="wgt", bufs=1) as wgtp,
            tc.tile_pool(name="xs", bufs=1) as xsp,
            tc.tile_pool(name="vo", bufs=1) as vop,
            tc.tile_pool(name="wrk", bufs=1) as wrk,
            tc.tile_pool(name="ps1", bufs=1, space="PSUM") as ps1,
            tc.tile_pool(name="ps2", bufs=1, space="PSUM") as ps2,
            tc.tile_pool(name="dram", bufs=1, space="DRAM") as dramp,
        ):
            P1 = ["pA", "pB", "pC", "pD"]  # 1-bank rotating psum tags

            # ---------------- constants ----------------
            ident = constp.tile([128, 128], f32, name="ident")
            make_identity(nc, ident[:])
            bias_sb = constp.tile([128, KT], f32, name="bias_sb")
            nc.sync.dma_start(bias_sb[:], bias_d[:])
            tmpv_sb = constp.tile([128, KT], f32, name="tmpv_sb")
            nc.sync.dma_start(tmpv_sb[:], tmpv_d[:])
            ones_sb = constp.tile([128, 1], f32, name="ones_sb")
            nc.vector.memset(ones_sb[:], 1.0)

            # first token super-tile of x^T, loaded ahead of the weights
            xst0 = []
            for kt in range(KT):
                t = xsp.tile([128, 512], f32r, tag=f"xs{kt}", bufs=2,
                             name=f"xa{kt}_0")
                nc.sync.dma_start(t[:], xT_d[kt * 128:(kt + 1) * 128, 0:512])
                xst0.append(t)
            # qk weight chunks, resident through A1
            wqk = []
            for kt in range(KT):
                w = wgtp.tile([128, 2 * C], f32r, tag=f"w{kt}", name=f"wqk{kt}")
                nc.sync.dma_start(w[:], wqkT_d[kt * 128:(kt + 1) * 128, :])
                wqk.append(w)

            # Gram accumulators: stA = heads 0,1 / stB = heads 2,3
            stA = ps2.tile([128, 1024], f32, tag="pE", name="stA")
            stB = ps2.tile([128, 1024], f32, tag="pF", name="stB")

            def st_slice(h, m):
                t = stA if h < 2 else stB
                off = (h % 2) * 512 + m * 256
                return t[:, off:off + 256]

            accq = wrk.tile([128, C], f32, tag="accq", name="accq")
            acck = wrk.tile([128, C], f32, tag="acck", name="acck")

            # ---------------- phase A1: q,k + Grams + sumsq ----------------
            for n5 in range(NT5):
                if n5 == 0:
                    xst = xst0
                else:
                    xst = []
                    for kt in range(KT):
                        t = xsp.tile([128, 512], f32r, tag=f"xs{kt}", bufs=2,
                                     name=f"xa{kt}_{n5}")
                        nc.sync.dma_start(
                            t[:], xT_d[kt * 128:(kt + 1) * 128,
                                       n5 * 512:(n5 + 1) * 512])
                        xst.append(t)
                for s in range(4):
                    tidx = n5 * 4 + s
                    qp0 = ps1.tile([128, 512], f32, tag="pA", name="qp0")
                    qp1 = ps1.tile([128, 512], f32, tag="pB", name="qp1")
                    kp0 = ps1.tile([128, 512], f32, tag="pC", name="kp0")
                    kp1 = ps1.tile([128, 512], f32, tag="pD", name="kp1")
                    for kt in range(KT):
                        lhs = xst[kt][:, s * 128:(s + 1) * 128]
                        fl, ll = (kt == 0), (kt == KT - 1)
                        nc.tensor.matmul(qp0[:], lhs, wqk[kt][:, 0:512],
                                         start=fl, stop=ll)
                        nc.tensor.matmul(qp1[:], lhs, wqk[kt][:, 512:1024],
                                         start=fl, stop=ll)
                        nc.tensor.matmul(kp0[:], lhs, wqk[kt][:, 1024:1536],
                                         start=fl, stop=ll)
                        nc.tensor.matmul(kp1[:], lhs, wqk[kt][:, 1536:2048],
                                         start=fl, stop=ll)
                    qcol = wrk.tile([128, C], f32r, tag="qcol", name="qcol")
                    kcol = wrk.tile([128, C], f32r, tag="kcol", name="kcol")
                    nc.vector.tensor_copy(qcol[:, 0:512], qp0[:])
                    nc.vector.tensor_copy(qcol[:, 512:1024], qp1[:])
                    nc.vector.tensor_copy(kcol[:, 0:512], kp0[:])
                    nc.vector.tensor_copy(kcol[:, 512:1024], kp1[:])
                    sq = wrk.tile([128, C], f32, tag="sq", name="sq")
                    sk = wrk.tile([128, C], f32, tag="sk", name="sk")
                    # square from the SBUF copies so the psum banks free
                    # after a single reader (keeps PE accumulation rolling)
                    nc.scalar.square(sq[:], qcol[:].bitcast(f32))
                    nc.scalar.square(sk[:], kcol[:].bitcast(f32))
                    if tidx == 0:
                        nc.gpsimd.tensor_copy(accq[:], sq[:])
                        nc.gpsimd.tensor_copy(acck[:], sk[:])
                    else:
                        nc.gpsimd.tensor_add(accq[:], accq[:], sq[:])
                        nc.gpsimd.tensor_add(acck[:], acck[:], sk[:])
                    for h in range(H):
                        for m in range(2):
                            nc.tensor.matmul(
                                st_slice(h, m),
                                kcol[:, h * 256 + m * 128: h * 256 + (m + 1) * 128],
                                qcol[:, h * 256:(h + 1) * 256],
                                start=(tidx == 0), stop=(tidx == 15),
                                skip_group_check=True)

            # sumsq rows: [1, 512] ones-matmuls into the freed qk psum slots
            ss_ps = []
            for i, (src, lo) in enumerate([(accq, 0), (accq, 512),
                                           (acck, 0), (acck, 512)]):
                sp = ps1.tile([1, 512], f32, tag=P1[i], name=f"ss{i}")
                nc.tensor.matmul(sp[:], ones_sb[:], src[:, lo:lo + 512],
                                 start=True, stop=True)
                ss_ps.append(sp)

            # SBUF bounces for the collective input (DMA cannot read PSUM);
            # all land in slots whose previous tenants just died.
            stA_sb = wrk.tile([128, 1024], f32, tag="qcol", name="stA_sb")
            stB_sb = wrk.tile([128, 1024], f32, tag="kcol", name="stB_sb")
            nc.vector.tensor_copy(stA_sb[:], stA[:])
            nc.vector.tensor_copy(stB_sb[:], stB[:])
            ss_sb = []
            for i, tg in enumerate(["sq", "sk", "accq", "acck"]):
                sb = wrk.tile([1, 512], f32, tag=tg, name=f"ssb{i}")
                nc.vector.tensor_copy(sb[:], ss_ps[i][:])
                ss_sb.append(sb)

            # ---------------- AllReduce over batch-pairs ----------------
            CCN = 128 * 2048 + 2 * C
            cc_in = dramp.tile([CCN], f32, name="cc_in")
            cc_out = dramp.tile([CCN], f32, name="cc_out")
            nc.sync.dma_start(
                cc_in[0:131072].rearrange("(p f) -> p f", p=128), stA_sb[:])
            nc.sync.dma_start(
                cc_in[131072:262144].rearrange("(p f) -> p f", p=128), stB_sb[:])
            for i in range(4):
                nc.sync.dma_start(
                    cc_in[262144 + i * 512: 262144 + (i + 1) * 512]
                    .rearrange("(a f) -> a f", a=1), ss_sb[i][:])
            nc.gpsimd.collective_compute(
                "AllReduce", ADD,
                replica_groups=[[0, 1], [2, 3], [4, 5], [6, 7]],
                ins=[cc_in.opt()], outs=[cc_out.opt()])
            strA = wrk.tile([128, 1024], f32, tag="qcol", name="strA")
            strB = wrk.tile([128, 1024], f32, tag="kcol", name="strB")
            nc.sync.dma_start(
                strA[:], cc_out[0:131072].rearrange("(p f) -> p f", p=128))
            nc.sync.dma_start(
                strB[:], cc_out[131072:262144].rearrange("(p f) -> p f", p=128))
            ssred = constp.tile([128, 16], f32, name="ssred")
            nc.sync.dma_start(
                ssred[:],
                cc_out[262144:262144 + 2048].rearrange("(j p) -> p j", p=128))

            def str_slice(h, m):
                t = strA if h < 2 else strB
                off = (h % 2) * 512 + m * 256
                return t[:, off:off + 256]

            # ---------------- phase A2: v (overlaps the collective) -------
            wv = []
            for kt in range(KT):
                w = wgtp.tile([128, C], f32r, tag=f"w{kt}", name=f"wv{kt}")
                nc.sync.dma_start(w[:], wvT_d[kt * 128:(kt + 1) * 128, :])
                wv.append(w)
            v_sb = [vop.tile([128, NL], f32r, tag=f"vo{cv}", name=f"v{cv}")
                    for cv in range(8)]
            pcnt = 0
            for pb in range(2):
                xst = []
                for kt in range(KT):
                    ta = xsp.tile([128, 512], f32r, tag=f"xs{kt}", bufs=2,
                                  name=f"xva{kt}_{pb}")
                    tb = xsp.tile([128, 512], f32r, tag=f"xs{kt}", bufs=2,
                                  name=f"xvb{kt}_{pb}")
                    nc.sync.dma_start(
                        ta[:], xT_d[kt * 128:(kt + 1) * 128,
                                    pb * 1024: pb * 1024 + 512])
                    nc.sync.dma_start(
                        tb[:], xT_d[kt * 128:(kt + 1) * 128,
                                    pb * 1024 + 512: pb * 1024 + 1024])
                    xst.append((ta, tb))
                for cv in range(8):
                    va = ps1.tile([128, 512], f32, tag=P1[pcnt % 4], name="vpa")
                    pcnt += 1
                    vb = ps1.tile([128, 512], f32, tag=P1[pcnt % 4], name="vpb")
                    pcnt += 1
                    for kt in range(KT):
                        fl, ll = (kt == 0), (kt == KT - 1)
                        nc.tensor.matmul(va[:],
                                         wv[kt][:, cv * 128:(cv + 1) * 128],
                                         xst[kt][0][:], start=fl, stop=ll)
                        nc.tensor.matmul(vb[:],
                                         wv[kt][:, cv * 128:(cv + 1) * 128],
                                         xst[kt][1][:], start=fl, stop=ll)
                    nc.vector.tensor_copy(
                        v_sb[cv][:, pb * 1024: pb * 1024 + 512], va[:])
                    nc.vector.tensor_copy(
                        v_sb[cv][:, pb * 1024 + 512: pb * 1024 + 1024], vb[:])

            # ---------------- phase B: normalize + softmax + attn@v -------
            # rq = temp/max(sqrt(ssq),eps), rk = 1/max(sqrt(ssk),eps), as
            # per-partition columns [128, 16]: cols 0-7 = rq, 8-15 = rk.
            rqk = constp.tile([128, 16], f32, name="rqk")
            nc.scalar.sqrt(rqk[:], ssred[:])
            nc.vector.tensor_scalar_max(rqk[:], rqk[:], EPS)
            nc.vector.reciprocal(rqk[:], rqk[:])
            nc.vector.tensor_mul(rqk[:, 0:8], rqk[:, 0:8], tmpv_sb[:])

            outT = []
            for h in range(H):
                # Gram^T rows d scaled by rk[d]
                sth = xsp.tile([128, 512], f32, tag="xs4", bufs=2, name="sth")
                for m in range(2):
                    nc.vector.tensor_scalar_mul(
                        sth[:, m * 256:(m + 1) * 256], str_slice(h, m),
                        rqk[:, 8 + 2 * h + m: 9 + 2 * h + m])
                # transpose to S[c, d]
                spm = ps1.tile([128, 512], f32, tag="pA", name="spm")
                for mc in range(2):
                    for md in range(2):
                        nc.tensor.transpose(
                            spm[:, mc * 256 + md * 128: mc * 256 + (md + 1) * 128],
                            sth[:, md * 256 + mc * 128: md * 256 + (mc + 1) * 128],
                            ident[:])
                sft = xsp.tile([128, 512], f32, tag="xs5", bufs=2, name="sft")
                for mc in range(2):
                    nc.vector.tensor_scalar_mul(
                        sft[:, mc * 256:(mc + 1) * 256],
                        spm[:, mc * 256:(mc + 1) * 256],
                        rqk[:, 2 * h + mc: 1 + 2 * h + mc])
                # softmax over d (free axis)
                negmax = wrk.tile([128, 2], f32, tag="negmax", name="negmax")
                rowsum = wrk.tile([128, 2], f32, tag="rowsum", name="rowsum")
                recip = wrk.tile([128, 2], f32, tag="recip", name="recip")
                esb = xsp.tile([128, 512], f32, tag="xs6", bufs=2, name="esb")
                for mc in range(2):
                    nc.vector.reduce_max(negmax[:, mc:mc + 1],
                                         sft[:, mc * 256:(mc + 1) * 256],
                                         axis=AX, negate=True)
                    nc.scalar.activation(esb[:, mc * 256:(mc + 1) * 256],
                                         sft[:, mc * 256:(mc + 1) * 256],
                                         Exp, bias=negmax[:, mc:mc + 1],
                                         accum_out=rowsum[:, mc:mc + 1])
                nc.vector.reciprocal(recip[:], rowsum[:])
                # attn^T (columns d on partitions)
                atp = ps1.tile([128, 512], f32, tag="pB", name="atp")
                for md in range(2):
                    for mc in range(2):
                        nc.tensor.transpose(
                            atp[:, md * 256 + mc * 128: md * 256 + (mc + 1) * 128],
                            esb[:, mc * 256 + md * 128: mc * 256 + (md + 1) * 128],
                            ident[:])
                atn = xsp.tile([128, 512], f32r, tag="xs7", bufs=2, name="atn")
                nc.vector.tensor_copy(atn[:], atp[:])
                # out^T[c,:] = sum_d attn^T[d,c] v[d,:], row-scaled by 1/rowsum
                op2s = {}
                for mc in range(2):
                    for nfh in range(2):
                        op2 = ps2.tile([128, 1024], f32,
                                       tag=("pE" if nfh == 0 else "pF"),
                                       name="op2")
                        op2s[(mc, nfh)] = op2
                        for md in range(2):
                            for n2 in range(2):
                                nc.tensor.matmul(
                                    op2[:, n2 * 512:(n2 + 1) * 512],
                                    atn[:, md * 256 + mc * 128: md * 256 + (mc + 1) * 128],
                                    v_sb[2 * h + md][:, nfh * 1024 + n2 * 512:
                                                     nfh * 1024 + (n2 + 1) * 512],
                                    start=(md == 0), stop=(md == 1))
                for mc in range(2):
                    i = 2 * h + mc
                    ot = vop.tile([128, NL], f32r, tag=f"vo{(i + 8) % 9}",
                                  name=f"ot{i}")
                    outT.append(ot)
                    for nfh in range(2):
                        nc.vector.tensor_scalar_mul(
                            ot[:, nfh * 1024:(nfh + 1) * 1024],
                            op2s[(mc, nfh)][:], recip[:, mc:mc + 1])

            # ---------------- phase C: projection + bias + residual -------
            wp = []
            for kt in range(KT):
                w = wgtp.tile([128, C], f32r, tag=f"w{kt}", name=f"wp{kt}")
                nc.sync.dma_start(w[:], wpT_d[kt * 128:(kt + 1) * 128, :])
                wp.append(w)
            for j in range(KT):
                pq = []
                for q in range(4):
                    p = ps1.tile([128, 512], f32, tag=P1[q], name=f"pp{q}")
                    pq.append(p)
                for kt in range(KT):
                    # proj input channel chunk kt = (jj=kt//2, d-half=kt%2);
                    # column block q is head q; tokens subsampled jj::4
                    for q in range(4):
                        nc.tensor.matmul(
                            pq[q][:],
                            wp[kt][:, j * 128:(j + 1) * 128],
                            outT[2 * q + kt % 2][:, (kt // 2)::4],
                            start=(kt == 0), stop=(kt == KT - 1))
                for q in range(4):
                    xr = xsp.tile([128, 512], f32r, tag=f"xs{4 + q}", bufs=2,
                                  name=f"xr{j}_{q}")
                    nc.sync.dma_start(
                        xr[:], xrT_d[j * 128:(j + 1) * 128,
                                     q * 512:(q + 1) * 512])
                    yq = xsp.tile([128, 512], f32, tag=f"xs{q}", bufs=2,
                                  name=f"yq{j}_{q}")
                    nc.scalar.activation(yq[:], pq[q][:], Ident,
                                         bias=bias_sb[:, j:j + 1])
                    nc.vector.tensor_add(yq[:], yq[:], xr[:].bitcast(f32))
                    nc.sync.dma_start(
                        yT_d[j * 128:(j + 1) * 128, q * 512:(q + 1) * 512],
                        yq[:])

    nc.compile()
    return nc


def _get_nc():
    if "nc" not in _CACHE:
        _CACHE["nc"] = _build()
    return _CACHE["nc"]


def _make_in_maps(x, Wqkv, Wproj, bproj, temperature):
    x = np.ascontiguousarray(np.asarray(x, dtype=np.float32))
    Wqkv = np.asarray(Wqkv, dtype=np.float32)
    Wproj = np.asarray(Wproj, dtype=np.float32)
    bproj = np.asarray(bproj, dtype=np.float32).reshape(C)
    temp = np.asarray(temperature, dtype=np.float32).reshape(H)

    WqkvT = np.ascontiguousarray(Wqkv.T)          # [C, 3C]
    wqkT = np.ascontiguousarray(WqkvT[:, :2 * C])
    wvT = np.ascontiguousarray(WqkvT[:, 2 * C:])
    wpT = np.ascontiguousarray(Wproj.T)
    bias2d = np.ascontiguousarray(bproj.reshape(KT, 128).T)
    tmpv2d = np.ascontiguousarray(np.repeat(temp, HD).reshape(KT, 128).T)

    in_maps = []
    for core in range(NCORES):
        b, half = core // 2, core % 2
        xT = np.ascontiguousarray(x[b, half * NL:(half + 1) * NL, :].T)
        rows = _out_rows(half)
        xrT = np.ascontiguousarray(x[b, rows, :].T)
        in_maps.append(dict(xT=xT, xrT=xrT, wqkT=wqkT, wvT=wvT, wpT=wpT,
                            bias=bias2d, tmpv=tmpv2d))
    return in_maps


def _out_rows(half):
    # torch transpose+reshape scramble: this core's y rows
    return np.concatenate(
        [h * 1024 + half * 512 + np.arange(512) for h in range(H)])


def _run(in_maps, trace=False, **kw):
    from concourse.bass_utils import run_bass_kernel_spmd

    nc = _get_nc()
    return run_bass_kernel_spmd(nc, in_maps, core_ids=list(range(NCORES)),
                                trace=trace, **kw)


def kernel(x, Wqkv, Wproj, bproj, temperature):
    res = _run(_make_in_maps(x, Wqkv, Wproj, bproj, temperature))
    y = np.empty((B, N, C), dtype=np.float32)
    for core in range(NCORES):
        b, half = core // 2, core % 2
        y[b, _out_rows(half), :] = res.results[core]["yT"].T
    return y



# revision 10
# speedup vs baseline: 1.8382x; 1.8382x over previous
"""Efficient Channel Attention kernel for 8 Trainium2 NeuronCores.

Problem (B=4, N=4096, C=1024, H=4, HD=256):
    qkv = x @ Wqkv.T                 -> q,k,v per head, [HD, N] layout
    q,k l2-normalized over N; scores = (q*temp) @ k.T   [HD, HD] per (b,h)
    attn = softmax(scores, -1); out = attn @ v; y = out @ Wproj.T + bproj + x

Sharding: core = (batch b, token-half); only cross-core data is the
token-contracted Grams k^T q + q/k sumsq, AllReduce'd (f32, ~1MB) within
the core pair sharing a batch.

All heavy GEMMs run as fp8e4m3 DoubleRow matmuls (2 slab-contraction per
pass = 157 TF/s, 2x the f32r rate). Host pre-quantizes x to fp8 and the
weights to fp8*64 (keeps W entries out of fp8 subnormals); scales unwind in
the psum->SBUF copies. Local tokens are host-permuted t -> (t%4)*512 + t//4
so the torch transpose+reshape channel scramble makes the proj-phase moving
operands contiguous (the f32r baseline paid 1.65x for stride-4 reads).
Residual + y I/O in bf16.

PSUM tags: q0,q1,k0,k1 (4x1 bank) + gA,gB (2x2 banks) cover all phases.
"""

import numpy as np

B, N, C, H = 4, 4096, 1024, 4
HD = C // H          # 256
NCORES = 8
NL = N // 2          # 2048 tokens per core
S = C // 128         # 8 channel slabs
NT = NL // 128       # 16 token tiles
EPS = 1e-12
WS = 64.0            # host weight scale (fp8 subnormal dodge)
CCN = 128 * 2048 + 2 * C

_CACHE = {}


def _build():
    import concourse.mybir as mybir
    import concourse.tile as tile
    from concourse import bacc
    from concourse.masks import make_identity

    f32 = mybir.dt.float32
    bf16 = mybir.dt.bfloat16
    f8 = mybir.dt.float8e4
    AX = mybir.AxisListType.X
    ADD = mybir.AluOpType.add
    MULT = mybir.AluOpType.mult
    DR = mybir.MatmulPerfMode.DoubleRow
    Exp = mybir.ActivationFunctionType.Exp
    Ident = mybir.ActivationFunctionType.Identity
    Sqrt = mybir.ActivationFunctionType.Sqrt

    nc = bacc.Bacc("TRN2", target_bir_lowering=False, debug=False,
                   num_devices=NCORES)

    x8_d = nc.dram_tensor("x8", [128, S, NL], f8, kind="ExternalInput").ap()
    wqk_d = nc.dram_tensor("wqk8", [128, S, 2 * C], f8, kind="ExternalInput").ap()
    wv_d = nc.dram_tensor("wv8", [128, S, C], f8, kind="ExternalInput").ap()
    wp_d = nc.dram_tensor("wp8", [128, S, C], f8, kind="ExternalInput").ap()
    xr_d = nc.dram_tensor("xr", [C, NL], bf16, kind="ExternalInput").ap()
    bias_d = nc.dram_tensor("bias", [128, S], f32, kind="ExternalInput").ap()
    tmpv_d = nc.dram_tensor("tmpv", [128, S], f32, kind="ExternalInput").ap()
    yT_d = nc.dram_tensor("yT", [C, NL], bf16, kind="ExternalOutput").ap()

    with tile.TileContext(nc) as tc:
        with (
            tc.tile_pool(name="const", bufs=1) as constp,
            tc.tile_pool(name="big", bufs=1) as bigp,
            tc.tile_pool(name="wrk", bufs=1) as wrk,
            tc.tile_pool(name="ps", bufs=1, space="PSUM") as ps,
            tc.tile_pool(name="ps2", bufs=1, space="PSUM") as ps2,
            tc.tile_pool(name="dram", bufs=1, space="DRAM") as dramp,
        ):
            PT = ["q0", "q1", "k0", "k1"]

            # ---------------- constants ----------------
            ident = constp.tile([128, 128], f32, name="ident")
            make_identity(nc, ident[:])
            bias_sb = constp.tile([128, S], f32, name="bias_sb")
            nc.sync.dma_start(bias_sb[:], bias_d[:])
            tmpv_sb = constp.tile([128, S], f32, name="tmpv_sb")
            nc.sync.dma_start(tmpv_sb[:], tmpv_d[:])
            ones_sb = constp.tile([128, 1], f32, name="ones_sb")
            nc.vector.memset(ones_sb[:], 1.0)

            # resident inputs; first slabs of x/wqk land first so A1 starts
            x8 = bigp.tile([128, S, NL], f8, name="x8")
            wqk = bigp.tile([128, S, 2 * C], f8, name="wqk")
            for s in range(S):
                nc.sync.dma_start(x8[:, s, :], x8_d[:, s, :])
                nc.sync.dma_start(wqk[:, s, :], wqk_d[:, s, :])
            wv = bigp.tile([128, S, C], f8, name="wv")
            wp = bigp.tile([128, S, C], f8, name="wp")
            xr = bigp.tile([128, S, NL], bf16, name="xr")
            for s in range(S):
                nc.sync.dma_start(wv[:, s, :], wv_d[:, s, :])
            for s in range(S):
                nc.sync.dma_start(wp[:, s, :], wp_d[:, s, :])
            for s in range(S):
                nc.sync.dma_start(xr[:, s, :], xr_d[s * 128:(s + 1) * 128, :])

            # Gram accumulators: stA = heads 0,1 / stB = heads 2,3
            stA = ps2.tile([128, 1024], f32, tag="gA", name="stA")
            stB = ps2.tile([128, 1024], f32, tag="gB", name="stB")

            def st_slice(h, m):
                t = stA if h < 2 else stB
                off = (h % 2) * 512 + m * 256
                return t[:, off:off + 256]

            accq = wrk.tile([128, C], f32, tag="accq", name="accq")
            acck = wrk.tile([128, C], f32, tag="acck", name="acck")

            # ---------------- phase A1: q,k + Grams + sumsq ----------------
            for tp in range(NT // 2):
                qcol = wrk.tile([128, 2, C], f8, tag="qcol", bufs=2,
                                name=f"qcol{tp}")
                kcol = wrk.tile([128, 2, C], f8, tag="kcol", bufs=2,
                                name=f"kcol{tp}")
                for i2 in range(2):
                    tidx = tp * 2 + i2
                    t0 = tidx * 128
                    qp0 = ps.tile([128, 512], f32, tag="q0", name="qp0")
                    qp1 = ps.tile([128, 512], f32, tag="q1", name="qp1")
                    kp0 = ps.tile([128, 512], f32, tag="k0", name="kp0")
                    kp1 = ps.tile([128, 512], f32, tag="k1", name="kp1")
                    for kt2 in range(4):
                        lhs = x8[:, 2 * kt2:2 * kt2 + 2, t0:t0 + 128]
                        w2 = wqk[:, 2 * kt2:2 * kt2 + 2, :]
                        fl, ll = (kt2 == 0), (kt2 == 3)
                        nc.tensor.matmul(qp0[:], lhs, w2[:, :, 0:512],
                                         start=fl, stop=ll, perf_mode=DR)
                        nc.tensor.matmul(qp1[:], lhs, w2[:, :, 512:1024],
                                         start=fl, stop=ll, perf_mode=DR)
                        nc.tensor.matmul(kp0[:], lhs, w2[:, :, 1024:1536],
                                         start=fl, stop=ll, perf_mode=DR)
                        nc.tensor.matmul(kp1[:], lhs, w2[:, :, 1536:2048],
                                         start=fl, stop=ll, perf_mode=DR)
                    # fp8 copies for the Gram (vector); squares of the fp8
                    # values for the norms (scalar q / gpsimd k, from SBUF —
                    # gpsimd cannot touch PSUM), accumulate sumsq
                    nc.vector.tensor_scalar_mul(qcol[:, i2, 0:512], qp0[:],
                                                1.0 / WS)
                    nc.vector.tensor_scalar_mul(qcol[:, i2, 512:1024], qp1[:],
                                                1.0 / WS)
                    nc.vector.tensor_scalar_mul(kcol[:, i2, 0:512], kp0[:],
                                                1.0 / WS)
                    nc.vector.tensor_scalar_mul(kcol[:, i2, 512:1024], kp1[:],
                                                1.0 / WS)
                    sq = wrk.tile([128, C], f32, tag="sq", bufs=2,
                                  name=f"sq{tidx}")
                    sk = wrk.tile([128, C], f32, tag="sk", bufs=2,
                                  name=f"sk{tidx}")
                    nc.scalar.square(sq[:], qcol[:, i2, :])
                    nc.gpsimd.tensor_mul(sk[:], kcol[:, i2, :],
                                         kcol[:, i2, :])
                    if tidx == 0:
                        nc.vector.tensor_copy(accq[:], sq[:])
                        nc.gpsimd.tensor_copy(acck[:], sk[:])
                    else:
                        nc.vector.tensor_add(accq[:], accq[:], sq[:])
                        nc.gpsimd.tensor_add(acck[:], acck[:], sk[:])
                for h in range(H):
                    for m in range(2):
                        nc.tensor.matmul(
                            st_slice(h, m),
                            kcol[:, :, h * 256 + m * 128: h * 256 + (m + 1) * 128],
                            qcol[:, :, h * 256:(h + 1) * 256],
                            start=(tp == 0), stop=(tp == NT // 2 - 1),
                            perf_mode=DR, skip_group_check=True)

            # sumsq rows: [1, 512] ones-matmuls into the freed qk psum slots
            ss_ps = []
            for i, (src, lo) in enumerate([(accq, 0), (accq, 512),
                                           (acck, 0), (acck, 512)]):
                sp = ps.tile([1, 512], f32, tag=PT[i], name=f"ss{i}")
                nc.tensor.matmul(sp[:], ones_sb[:], src[:, lo:lo + 512],
                                 start=True, stop=True)
                ss_ps.append(sp)

            # SBUF bounces for the collective (DMA cannot read PSUM)
            # NOTE: bf16 collective hangs the runtime in-context (probed OK
            # bare); f32 collective is reliable — keep f32.
            ccdt = f32
            stA_sb = wrk.tile([128, 1024], ccdt, tag="ccA", name="stA_sb")
            stB_sb = wrk.tile([128, 1024], ccdt, tag="ccB", name="stB_sb")
            nc.vector.tensor_copy(stA_sb[:], stA[:])
            nc.vector.tensor_copy(stB_sb[:], stB[:])
            ss_sb = []
            for i in range(4):
                sb = wrk.tile([1, 512], ccdt, tag=f"ssb{i}", name=f"ssb{i}")
                nc.vector.tensor_copy(sb[:], ss_ps[i][:])
                ss_sb.append(sb)

            # ---------------- AllReduce over batch-pairs (bf16) -----------
            cc_in = dramp.tile([CCN], ccdt, name="cc_in")
            cc_out = dramp.tile([CCN], ccdt, name="cc_out")
            nc.sync.dma_start(
                cc_in[0:131072].rearrange("(p f) -> p f", p=128), stA_sb[:])
            nc.sync.dma_start(
                cc_in[131072:262144].rearrange("(p f) -> p f", p=128), stB_sb[:])
            for i in range(4):
                nc.sync.dma_start(
                    cc_in[262144 + i * 512: 262144 + (i + 1) * 512]
                    .rearrange("(a f) -> a f", a=1), ss_sb[i][:])
            nc.gpsimd.collective_compute(
                "AllReduce", ADD,
                replica_groups=[[0, 1], [2, 3], [4, 5], [6, 7]],
                ins=[cc_in.opt()], outs=[cc_out.opt()])
            strA = wrk.tile([128, 1024], ccdt, tag="ccA", name="strA")
            strB = wrk.tile([128, 1024], ccdt, tag="ccB", name="strB")
            nc.sync.dma_start(
                strA[:], cc_out[0:131072].rearrange("(p f) -> p f", p=128))
            nc.sync.dma_start(
                strB[:], cc_out[131072:262144].rearrange("(p f) -> p f", p=128))
            ssred = constp.tile([128, 16], ccdt, name="ssred")
            nc.sync.dma_start(
                ssred[:],
                cc_out[262144:262144 + 2048].rearrange("(j p) -> p j", p=128))

            def str_slice(h, m):
                t = strA if h < 2 else strB
                off = (h % 2) * 512 + m * 256
                return t[:, off:off + 256]

            # ---------------- phase A2: v (overlaps the collective) -------
            v_sb = [bigp.tile([128, 2, NL], f8, name=f"v{h}") for h in range(H)]
            for vb in range(8):
                h, iv = vb // 2, vb % 2
                if vb % 2 == 0:
                    vps = [ps.tile([128, 512], f32, tag=PT[tc],
                                   name=f"vp{vb}_{tc}")[:] for tc in range(4)]
                else:
                    vA = ps2.tile([128, 1024], f32, tag="gA", name=f"vA{vb}")
                    vB = ps2.tile([128, 1024], f32, tag="gB", name=f"vB{vb}")
                    vps = [vA[:, 0:512], vA[:, 512:1024],
                           vB[:, 0:512], vB[:, 512:1024]]
                for kt2 in range(4):
                    fl, ll = (kt2 == 0), (kt2 == 3)
                    wvs = wv[:, 2 * kt2:2 * kt2 + 2, vb * 128:(vb + 1) * 128]
                    for tc in range(4):
                        nc.tensor.matmul(
                            vps[tc], wvs,
                            x8[:, 2 * kt2:2 * kt2 + 2, tc * 512:(tc + 1) * 512],
                            start=fl, stop=ll, perf_mode=DR)
                for tc in range(4):
                    nc.vector.tensor_scalar_mul(
                        v_sb[h][:, iv, tc * 512:(tc + 1) * 512], vps[tc],
                        1.0 / WS)

            # ---------------- phase B: normalize + softmax + attn@v -------
            # rq = temp/max(||q||,eps), rk = 1/max(||k||,eps) per channel:
            # rqk [128, 16]: cols 0-7 = rq (chan j*128+p), 8-15 = rk.
            rqk = constp.tile([128, 16], f32, name="rqk")
            nc.scalar.activation(rqk[:], ssred[:], Sqrt)
            nc.vector.tensor_scalar_max(rqk[:], rqk[:], EPS)
            nc.vector.reciprocal(rqk[:], rqk[:])
            nc.vector.tensor_mul(rqk[:, 0:8], rqk[:, 0:8], tmpv_sb[:])

            outT = [bigp.tile([128, 2, NL], f8, name=f"ot{h}") for h in range(H)]
            # pass 1: per-head softmax -> fp8 attn^T (all heads before pass 2
            # so the scalar engine's activation table switches only once)
            atns, recips = [], []
            for h in range(H):
                # Gram^T rows d scaled by rk[d]
                sth = wrk.tile([128, 512], f32, tag="sth", bufs=2,
                               name=f"sth{h}")
                for m in range(2):
                    nc.vector.tensor_scalar_mul(
                        sth[:, m * 256:(m + 1) * 256], str_slice(h, m),
                        rqk[:, 8 + 2 * h + m: 9 + 2 * h + m])
                spm = ps.tile([128, 512], f32, tag="q0", name=f"spm{h}")
                for mc in range(2):
                    for md in range(2):
                        nc.tensor.transpose(
                            spm[:, mc * 256 + md * 128: mc * 256 + (md + 1) * 128],
                            sth[:, md * 256 + mc * 128: md * 256 + (mc + 1) * 128],
                            ident[:])
                sft = wrk.tile([128, 512], f32, tag="sft", bufs=2,
                               name=f"sft{h}")
                for mc in range(2):
                    nc.vector.tensor_scalar_mul(
                        sft[:, mc * 256:(mc + 1) * 256],
                        spm[:, mc * 256:(mc + 1) * 256],
                        rqk[:, 2 * h + mc: 1 + 2 * h + mc])
                negmax = wrk.tile([128, 2], f32, tag="negmax", bufs=2,
                                  name=f"nm{h}")
                rowsum = wrk.tile([128, 2], f32, tag="rowsum", bufs=2,
                                  name=f"rs{h}")
                recip = wrk.tile([128, 2], f32, tag=f"recip{h}",
                                 name=f"rc{h}")
                esb = wrk.tile([128, 512], f32, tag="esb", bufs=2,
                               name=f"esb{h}")
                for mc in range(2):
                    nc.vector.reduce_max(negmax[:, mc:mc + 1],
                                         sft[:, mc * 256:(mc + 1) * 256],
                                         axis=AX, negate=True)
                    nc.scalar.activation(esb[:, mc * 256:(mc + 1) * 256],
                                         sft[:, mc * 256:(mc + 1) * 256],
                                         Exp, bias=negmax[:, mc:mc + 1],
                                         accum_out=rowsum[:, mc:mc + 1])
                nc.vector.reciprocal(recip[:], rowsum[:])
                # fold the outT fp8 scale (x16) into the softmax denominator
                nc.vector.tensor_scalar_mul(recip[:], recip[:], 16.0)
                atp = ps.tile([128, 512], f32, tag="q1", name=f"atp{h}")
                for md in range(2):
                    for mc in range(2):
                        nc.tensor.transpose(
                            atp[:, md * 256 + mc * 128: md * 256 + (mc + 1) * 128],
                            esb[:, mc * 256 + md * 128: mc * 256 + (md + 1) * 128],
                            ident[:])
                atn = wrk.tile([128, 2, 256], f8, tag=f"atn{h}",
                               name=f"atn{h}")
                for i in range(2):
                    nc.vector.tensor_copy(atn[:, i, :],
                                          atp[:, i * 256:(i + 1) * 256])
                atns.append(atn)
                recips.append(recip)
            # pass 2: out^T[c,:] = sum_d attn^T[d,c] v[d,:] (one DoubleRow
            # pass); psum copy-scales split scalar/vector (gpsimd can't)
            cnt = 0
            for h in range(H):
                atn, recip = atns[h], recips[h]
                for mc in range(2):
                    for tc in range(4):
                        tg = ["k0", "k1", "gA", "gB"][cnt % 4]
                        pp = ps if cnt % 4 < 2 else ps2
                        op = pp.tile([128, 512], f32, tag=tg,
                                     name=f"op{h}_{mc}_{tc}")
                        cnt += 1
                        nc.tensor.matmul(
                            op[:], atn[:, :, mc * 128:(mc + 1) * 128],
                            v_sb[h][:, :, tc * 512:(tc + 1) * 512],
                            start=True, stop=True, perf_mode=DR)
                        dst = outT[h][:, mc, tc * 512:(tc + 1) * 512]
                        if tc % 2 == 0:
                            nc.scalar.activation(dst, op[:], Ident,
                                                 scale=recip[:, mc:mc + 1])
                        else:
                            nc.vector.tensor_scalar_mul(dst, op[:],
                                                        recip[:, mc:mc + 1])

            # ---------------- phase C: projection + bias + residual -------
            for j in range(S):
                if j % 2 == 0:
                    pq = [ps.tile([128, 512], f32, tag=PT[q],
                                  name=f"pq{j}_{q}")[:] for q in range(4)]
                else:
                    pA = ps2.tile([128, 1024], f32, tag="gA", name=f"pA{j}")
                    pB = ps2.tile([128, 1024], f32, tag="gB", name=f"pB{j}")
                    pq = [pA[:, 0:512], pA[:, 512:1024],
                          pB[:, 0:512], pB[:, 512:1024]]
                for kt2 in range(4):
                    fl, ll = (kt2 == 0), (kt2 == 3)
                    wps = wp[:, 2 * kt2:2 * kt2 + 2, j * 128:(j + 1) * 128]
                    for q in range(4):
                        nc.tensor.matmul(
                            pq[q], wps,
                            outT[q][:, :, kt2 * 512:(kt2 + 1) * 512],
                            start=fl, stop=ll, perf_mode=DR)
                for q in range(4):
                    yq = wrk.tile([128, 512], bf16, tag=f"yq{q % 2}", bufs=2,
                                  name=f"yq{j}_{q}")
                    if q % 2 == 0:
                        nc.scalar.activation(yq[:], pq[q], Ident,
                                             bias=bias_sb[:, j:j + 1],
                                             scale=1.0 / (WS * 16.0))
                    else:
                        nc.vector.tensor_scalar(yq[:], pq[q], 1.0 / (WS * 16.0),
                                                bias_sb[:, j:j + 1],
                                                op0=MULT, op1=ADD)
                    nc.gpsimd.tensor_add(yq[:], yq[:],
                                         xr[:, j, q * 512:(q + 1) * 512])
                    nc.sync.dma_start(
                        yT_d[j * 128:(j + 1) * 128, q * 512:(q + 1) * 512],
                        yq[:])

    nc.compile()
    return nc


def _get_nc():
    if "nc" not in _CACHE:
        _CACHE["nc"] = _build()
    return _CACHE["nc"]


def _out_rows(half):
    # torch transpose+reshape scramble: this core's y rows
    return np.concatenate(
        [h * 1024 + half * 512 + np.arange(512) for h in range(H)])


def _make_in_maps(x, Wqkv, Wproj, bproj, temperature):
    import ml_dtypes
    f8 = ml_dtypes.float8_e4m3
    bf = ml_dtypes.bfloat16

    x = np.ascontiguousarray(np.asarray(x, dtype=np.float32))
    Wqkv = np.asarray(Wqkv, dtype=np.float32)
    Wproj = np.asarray(Wproj, dtype=np.float32)
    bproj = np.asarray(bproj, dtype=np.float32).reshape(C)
    temp = np.asarray(temperature, dtype=np.float32).reshape(H)

    WqkvT = Wqkv.T                                # [C, 3C]
    wqk8 = (WqkvT[:, :2 * C] * WS).reshape(S, 128, 2 * C) \
        .transpose(1, 0, 2).astype(f8)
    wv8 = (WqkvT[:, 2 * C:] * WS).reshape(S, 128, C) \
        .transpose(1, 0, 2).astype(f8)
    wp8 = (Wproj.T * WS).reshape(S, 128, C).transpose(1, 0, 2).astype(f8)
    bias2d = np.ascontiguousarray(bproj.reshape(S, 128).T)
    tmpv2d = np.ascontiguousarray(np.repeat(temp, HD).reshape(S, 128).T)

    # store position p holds original local token t = 4*(p%512) + p//512 so
    # the proj-phase moving operands are contiguous
    tmap = 4 * (np.arange(NL) % 512) + np.arange(NL) // 512

    in_maps = []
    for core in range(NCORES):
        b, half = core // 2, core % 2
        xl = x[b, half * NL:(half + 1) * NL, :]   # [NL, C]
        x8 = xl[tmap, :].T.reshape(S, 128, NL).transpose(1, 0, 2).astype(f8)
        rows = _out_rows(half)
        xrb = x[b, rows, :].T.astype(bf)          # [C, NL]
        in_maps.append(dict(x8=x8, xr=np.ascontiguousarray(xrb),
                            wqk8=wqk8, wv8=wv8, wp8=wp8,
                            bias=bias2d, tmpv=tmpv2d))
    return in_maps


def _run(in_maps, trace=False, **kw):
    from concourse.bass_utils import run_bass_kernel_spmd

    nc = _get_nc()
    return run_bass_kernel_spmd(nc, in_maps, core_ids=list(range(NCORES)),
                                trace=trace, **kw)


def kernel(x, Wqkv, Wproj, bproj, temperature):
    res = _run(_make_in_maps(x, Wqkv, Wproj, bproj, temperature))
    y = np.empty((B, N, C), dtype=np.float32)
    for core in range(NCORES):
        b, half = core // 2, core % 2
        y[b, _out_rows(half), :] = res.results[core]["yT"].T.astype(np.float32)
    return y
